# revision 29
# baseline (speedup 1.0000x reference)
"""CCNet unit (conv3x3 -> BN/ReLU -> 2x criss-cross attention -> conv3x3 ->
BN/ReLU) on 8 Trainium2 NeuronCores.

Sharding (SPMD-symmetric program; per-core differences live only in data):
  core = 2*b + half   (b = sample 0..3, half = 0/1)
  - conv1 (Cin=2048): input-channel split across the pair; partial sums
    pair-AllReduce'd (fp16, in h-halves), then BN+ReLU on both cores.
  - CCA x2: computed redundantly by both pair members (cheap vs conv1).
  - conv2 (Cout=512): output-channel split via sharded weights.

All matmuls fp16 (full PE rate), PSUM accumulates fp32.

Attention layouts (parity-packed so matmul operand base partitions match):
  expT_e/o  [64, 2048] : exp(eH^T) at [i, (w//2)*64 + h] for even/odd w,
                         diag-masked, unnormalized
  expTW_e/o [64, 2048] : exp(eW^T) at [j, (h//2)*64 + w] for even/odd h
  expblkH   [128, 4096]: block-diag normalized exp for column pairs
                         (2t,2t+1): [0:64, 128t:128t+64] = even,
                         [64:128, 128t+64:128t+128] = odd, rest zero
  expblkW   [128, 4096]: same for row pairs
  vtW[t]    [128, 512] : v rows (2t,2t+1): partition 64*(h%2)+w
  vtH[t]    [128, 512] : v cols (2t,2t+1): partition 64*(w%2)+h
Aggregation matmuls consume (vt, expblk) pairs directly: [128,128] rhs
per pair -> psum [c, 8 rows/cols, 64], drained contiguously.
Softmax has no max-subtraction (logits bounded ~ +-5); normalization
(gamma/s) is folded into expblk; v-bias*gamma (gvb) added at the dir-W
drain; dir-H lands in a compact col-major buffer merged by one
transposed add per channel tile.
"""

import sys

sys.path.insert(0, "/opt/trn_rl_repo")

import numpy as np
import ml_dtypes

import concourse.bacc as bacc
import concourse.mybir as mybir
import concourse.tile as tile
from concourse.bass_utils import run_bass_kernel_spmd

dt = mybir.dt
AF = mybir.ActivationFunctionType
ALU = mybir.AluOpType

B, CIN, C, CR, H, W = 4, 2048, 512, 64, 64, 64
HW = H * W
WP = W + 2
PADPX = (H + 2) * WP
INT0 = WP + 1
KT1 = CIN // 2 // 128   # 8
CT = C // 128           # 4
N_CORES = 8
PAIRS = [[0, 1], [2, 3], [4, 5], [6, 7]]

DEBUG_STAGE = None
_COMPILED = {}


def _pad_hw(t, ct=None):
    """[128, 66, 66] padded view (or of channel-tile ct)."""
    v = t[:, ct, :] if ct is not None else t[:]
    return v.rearrange("p (h w) -> p h w", w=WP)


def _interior(t, ct=None):
    """[128, 64, 64] interior view."""
    return _pad_hw(t, ct)[:, 1:1 + H, 1:1 + W]


def _chunk_rhs(t, ct, h0, off):
    """[128, 8, 64] rhs AP: 8 output rows from h0, reading offset `off`."""
    r0, c0 = h0 + off // WP, off % WP
    v = _pad_hw(t, ct)
    return v[:, r0:r0 + 8, c0:c0 + 64]


def _memset_border(nc, t, ct=None, eng=None):
    """Zero only the padding border of a [*, (66,66)] padded tile."""
    eng = eng or nc.vector
    v = _pad_hw(t, ct)
    eng.memset(v[:, 0, :], 0.0)
    eng.memset(v[:, H + 1, :], 0.0)
    # (row r, col 65) and (row r+1, col 0) pairs, r = 0..64
    base = t[:, ct, :] if ct is not None else t[:]
    strip = base[:, W + 1:W + 1 + 65 * WP].rearrange(
        "p (r c) -> p r c", c=WP)[:, :, 0:2]
    eng.memset(strip, 0.0)


def build_kernel(debug_stage=None):
    nc = bacc.Bacc("TRN2", target_bir_lowering=False, debug=False,
                   num_devices=N_CORES)

    x_in = nc.dram_tensor("x", [KT1, 128, HW], dt.float16, kind="ExternalInput").ap()
    w1_in = nc.dram_tensor("w1", [KT1, CT, 9, 128, 128], dt.float16, kind="ExternalInput").ap()
    bn1_in = nc.dram_tensor("bn1", [128, CT, 2], dt.float32, kind="ExternalInput").ap()
    qkw_in = nc.dram_tensor("qkw", [CT, 128, 128], dt.float16, kind="ExternalInput").ap()
    qb_in = nc.dram_tensor("qb", [64, 1], dt.float32, kind="ExternalInput").ap()
    vw_in = nc.dram_tensor("vw", [CT, 128, C], dt.float16, kind="ExternalInput").ap()
    gvb_in = nc.dram_tensor("gvb", [128, CT], dt.float32, kind="ExternalInput").ap()
    og_in = nc.dram_tensor("og", [128, 1], dt.bfloat16, kind="ExternalInput").ap()
    mask_in = nc.dram_tensor("mask", [64, 64], dt.bfloat16, kind="ExternalInput").ap()
    w2_in = nc.dram_tensor("w2", [CT, 9, 128, 256], dt.float16, kind="ExternalInput").ap()
    bn2_in = nc.dram_tensor("bn2", [128, 2, 2], dt.float32, kind="ExternalInput").ap()

    if debug_stage is None:
        out_t = nc.dram_tensor("out", [2, 128, HW], dt.float32, kind="ExternalOutput").ap()
    else:
        out_t = nc.dram_tensor("out", [CT, 128, HW], dt.float32, kind="ExternalOutput").ap()

    with tile.TileContext(nc) as tc:
        _emit(nc, tc, debug_stage, x_in, w1_in, bn1_in, qkw_in, qb_in, vw_in,
              gvb_in, og_in, mask_in, w2_in, bn2_in, out_t)
    nc.compile()
    return nc


def _emit(nc, tc, debug_stage, x_in, w1_in, bn1_in, qkw_in, qb_in, vw_in,
          gvb_in, og_in, mask_in, w2_in, bn2_in, out_t):
    from contextlib import ExitStack

    ctx = ExitStack()
    with ctx:
        pool_feats = ctx.enter_context(tc.tile_pool(name="feats", bufs=1))
        featsA = pool_feats.tile([128, CT, PADPX], dt.float16)
        for ct in range(CT):
            _memset_border(nc, featsA, ct,
                           eng=nc.vector if ct % 2 == 0 else nc.gpsimd)

        pool_const = ctx.enter_context(tc.tile_pool(name="const", bufs=1))
        bn1 = pool_const.tile([128, CT, 2], dt.float32)
        qkw = pool_const.tile([128, CT, 128], dt.float16)
        qb = pool_const.tile([64, 1], dt.float32)
        vw = pool_const.tile([128, CT, C], dt.float16)
        gvb = pool_const.tile([128, CT], dt.float32)
        og = pool_const.tile([128, 1], dt.bfloat16)
        mask = pool_const.tile([64, 64], dt.bfloat16)
        ones1b = pool_const.tile([1, 128], dt.bfloat16)
        nc.vector.memset(ones1b[:], 1.0)
        nc.sync.dma_start(bn1[:], bn1_in[:])
        nc.sync.dma_start(qkw[:], qkw_in[:].rearrange("k p c -> p k c"))
        nc.sync.dma_start(qb[:], qb_in[:])
        nc.sync.dma_start(vw[:], vw_in[:].rearrange("k p c -> p k c"))
        nc.sync.dma_start(gvb[:], gvb_in[:])
        nc.sync.dma_start(og[:], og_in[:])
        nc.sync.dma_start(mask[:], mask_in[:])

        # ---------------- conv1 ----------------
        with (
            tc.tile_pool(name="c1x", bufs=1) as c1x,
            tc.tile_pool(name="c1w", bufs=1) as c1w,
            tc.tile_pool(name="c1ps", bufs=1, space="PSUM") as c1ps,
            tc.tile_pool(name="c1st", bufs=4) as c1st,
            tc.tile_pool(name="c1dram", bufs=1, space="DRAM") as c1dram,
        ):
            xts, wts = [], []
            for kt in range(KT1):
                wt = c1w.tile([128, 9, C], dt.float16, tag=f"w{kt}")
                xt = c1x.tile([128, PADPX], dt.float16, tag=f"x{kt}")
                _memset_border(nc, xt,
                               eng=nc.vector if kt % 2 == 0 else nc.gpsimd)
                xv = x_in[kt].rearrange("p (h w) -> p h w", w=W)
                iv = _interior(xt)
                if kt == 0:
                    # first chunks small so mt0/kt0 compute starts early
                    nc.sync.dma_start(wt[:, :, 0:128],
                                      w1_in[kt, 0].rearrange("t p c -> p t c"))
                    for rc in range(4):
                        nc.sync.dma_start(iv[:, rc * 16:(rc + 1) * 16, :],
                                          xv[:, rc * 16:(rc + 1) * 16, :])
                    for mt in range(1, CT):
                        nc.sync.dma_start(
                            wt[:, :, mt * 128:(mt + 1) * 128],
                            w1_in[kt, mt].rearrange("t p c -> p t c"))
                else:
                    for mt in range(CT):
                        nc.sync.dma_start(
                            wt[:, :, mt * 128:(mt + 1) * 128],
                            w1_in[kt, mt].rearrange("t p c -> p t c"))
                    nc.sync.dma_start(iv, xv)
                xts.append(xt)
                wts.append(wt)

            partial = c1dram.tile([CT, 4, 128, HW // 4], dt.float16)
            reduced = c1dram.tile([CT, 4, 128, HW // 4], dt.float16)

            def drain(mt, h8):
                st = c1st.tile([128, 512], dt.float16, tag="st",
                               name=f"st{mt}_{h8}")
                nc.scalar.activation(st[:], pss[h8][:], AF.Copy)
                nc.sync.dma_start(
                    partial[mt, h8 // 2, :, (h8 % 2) * 512:(h8 % 2 + 1) * 512],
                    st[:])

            def allreduce(mt, q0=0, q1=4):
                if q1 - q0 == 4:
                    nc.gpsimd.collective_compute(
                        "AllReduce", ALU.add, replica_groups=PAIRS,
                        ins=[partial[mt]], outs=[reduced[mt]])
                else:
                    nc.gpsimd.collective_compute(
                        "AllReduce", ALU.add, replica_groups=PAIRS,
                        ins=[partial[mt, q0:q1]], outs=[reduced[mt, q0:q1]])
                for h8 in range(q0 * 2, q1 * 2):
                    red_sb = c1st.tile([128, 512], dt.float16, tag="redsb",
                                       name=f"red{mt}_{h8}")
                    nc.sync.dma_start(
                        red_sb[:],
                        reduced[mt, h8 // 2, :,
                                (h8 % 2) * 512:(h8 % 2 + 1) * 512])
                    nc.scalar.activation(
                        _pad_hw(featsA, mt)[:, 1 + h8 * 8:1 + h8 * 8 + 8,
                                            1:1 + W],
                        red_sb[:].rearrange("p (h w) -> p h w", w=W),
                        AF.Relu, bias=bn1[:, mt, 1:2], scale=bn1[:, mt, 0:1])

            # mt=0: kt-outer so compute starts as soon as x[0]/w[0] land
            pss = [c1ps.tile([128, 512], dt.float32, tag=f"ps{h8}",
                             name=f"ps0_{h8}") for h8 in range(8)]
            for kt in range(KT1):
                for h8 in range(8):
                    for tap in range(9):
                        dy, dx = tap // 3, tap % 3
                        nc.tensor.matmul(
                            pss[h8][:],
                            wts[kt][:, tap, 0:128],
                            _pad_hw(xts[kt])[:, h8 * 8 + dy:h8 * 8 + dy + 8,
                                             dx:dx + 64],
                            start=(kt == 0 and tap == 0),
                            stop=(kt == KT1 - 1 and tap == 8))
            for h8 in range(8):
                drain(0, h8)
            allreduce(0)

            # mt=1..3: h0-outer (streaming drains, early AllReduce issue)
            for mt in range(1, CT):
                pss = [c1ps.tile([128, 512], dt.float32, tag=f"ps{h8}",
                                 name=f"ps{mt}_{h8}") for h8 in range(8)]
                for h8 in range(8):
                    i = 0
                    for kt in range(KT1):
                        for tap in range(9):
                            dy, dx = tap // 3, tap % 3
                            nc.tensor.matmul(
                                pss[h8][:],
                                wts[kt][:, tap, mt * 128:(mt + 1) * 128],
                                _pad_hw(xts[kt])[:, h8 * 8 + dy:h8 * 8 + dy + 8,
                                                 dx:dx + 64],
                                start=(i == 0), stop=(i == KT1 * 9 - 1))
                            i += 1
                    drain(mt, h8)
                    if mt == CT - 1 and h8 == 3:
                        allreduce(mt, 0, 2)   # overlaps h8=4..7 compute
                    if mt == CT - 1 and h8 == 5:
                        allreduce(mt, 2, 3)   # overlaps h8=6..7 compute
                if mt < CT - 1:
                    allreduce(mt)
                else:
                    allreduce(mt, 3, 4)

        if debug_stage == "feats1":
            _emit_debug_out(nc, tc, featsA, out_t)
            return

        # ---------------- CCA x2 ----------------
        pool_fb = ctx.enter_context(tc.tile_pool(name="featsB", bufs=1))
        featsB = pool_fb.tile([128, CT, PADPX], dt.float16)
        pool_blk = ctx.enter_context(tc.tile_pool(name="expblk", bufs=1))
        expblkH = pool_blk.tile([128, 4096], dt.bfloat16)
        expblkW = pool_blk.tile([128, 4096], dt.bfloat16)
        nc.vector.memset(expblkH[:], 0.0)
        nc.gpsimd.memset(expblkW[:], 0.0)
        for ct in range(CT):
            _memset_border(nc, featsB, ct,
                           eng=nc.vector if ct % 2 == 0 else nc.gpsimd)
        _emit_cca(nc, tc, featsA, featsB, qkw, qb, vw, gvb, og, mask,
                  ones1b, expblkH, expblkW)
        if debug_stage == "cca1":
            _emit_debug_out(nc, tc, featsB, out_t)
            return
        pool_c2 = ctx.enter_context(tc.tile_pool(name="c2w", bufs=1))
        w2 = pool_c2.tile([128, CT, 9, 256], dt.float16)
        bn2 = pool_c2.tile([128, 2, 2], dt.float32)
        nc.sync.dma_start(bn2[:], bn2_in[:])
        for kt in range(CT):
            nc.sync.dma_start(w2[:, kt, :, :],
                              w2_in[kt].rearrange("t p c -> p t c"))

        _emit_cca(nc, tc, featsB, featsA, qkw, qb, vw, gvb, og, mask,
                  ones1b, expblkH, expblkW)
        if debug_stage == "cca2":
            _emit_debug_out(nc, tc, featsA, out_t)
            return

        # ---------------- conv2 ----------------
        with (
            tc.tile_pool(name="c2ps", bufs=8, space="PSUM") as c2ps,
            tc.tile_pool(name="c2st", bufs=4) as c2st,
        ):
            for mt in range(2):
                for h0 in range(0, H, 8):
                    ps = c2ps.tile([128, 512], dt.float32)
                    i = 0
                    for kt in range(CT):
                        for dy in range(3):
                            for dx in range(3):
                                nc.tensor.matmul(
                                    ps[:],
                                    w2[:, kt, dy * 3 + dx, mt * 128:(mt + 1) * 128],
                                    _chunk_rhs(featsA, kt, h0, dy * WP + dx),
                                    start=(i == 0), stop=(i == CT * 9 - 1))
                                i += 1
                    st = c2st.tile([128, 512], dt.float32)
                    nc.scalar.activation(st[:], ps[:], AF.Relu,
                                         bias=bn2[:, mt, 1:2],
                                         scale=bn2[:, mt, 0:1])
                    nc.sync.dma_start(out_t[mt, :, h0 * W:(h0 + 8) * W], st[:])


def _emit_debug_out(nc, tc, feats, out_t):
    with tc.tile_pool(name="dbg", bufs=4) as dbg:
        for ct in range(CT):
            st = dbg.tile([128, HW], dt.float32)
            nc.vector.tensor_copy(
                st[:].rearrange("p (h w) -> p h w", w=W), _interior(feats, ct))
            nc.sync.dma_start(out_t[ct], st[:])


def _emit_cca(nc, tc, fin, fout, qkw, qb, vw, gvb, og, mask, ones1b,
              expblkH, expblkW):
    """fout = gamma*cca(fin) + fin (interior; fout border must be zero)."""
    from contextlib import ExitStack

    ctx = ExitStack()
    with ctx:
        sb = ctx.enter_context(tc.tile_pool(name="cca_sb", bufs=1))
        vpool = ctx.enter_context(tc.tile_pool(name="cca_v", bufs=1))
        psp = ctx.enter_context(tc.tile_pool(name="cca_ps", bufs=1, space="PSUM"))

        def blk_view(blk, par, g):
            v = blk[64 * par:64 * (par + 1), :].rearrange(
                "p (t x) -> p t x", x=128)
            return v[:, 4 * g:4 * (g + 1), 64 * par:64 * (par + 1)]

        vt = [None] * 32
        with tc.tile_pool(name="cca_frm", bufs=1) as frmp:
            frm = frmp.tile([128, CT, HW], dt.float16)
            with (
                tc.tile_pool(name="cca_qk", bufs=1) as qkp,
                tc.tile_pool(name="cca_small", bufs=2) as sp,
            ):
                for ct in range(CT):
                    nc.sync.dma_start(
                        frm[:, ct, :].rearrange("p (h w) -> p h w", w=W),
                        _interior(fin, ct))

                # ---- q, k projections (separate half-width matmuls so
                # both drain at partition base 0; no DMA shuffles) ----
                q_sb = qkp.tile([64, HW], dt.float16)
                k_sb = qkp.tile([64, HW], dt.float16)
                for n in range(8):
                    psq = psp.tile([64, 512], dt.float32, tag="small",
                                   bufs=4, name=f"psq{n}")
                    psk = psp.tile([64, 512], dt.float32, tag="small",
                                   bufs=4, name=f"psk{n}")
                    for kt in range(CT):
                        rhs = _chunk_rhs(fin, kt, n * 8, INT0)
                        nc.tensor.matmul(psq[:], qkw[:, kt, 0:64], rhs,
                                         start=(kt == 0), stop=(kt == CT - 1))
                        nc.tensor.matmul(psk[:], qkw[:, kt, 64:128], rhs,
                                         start=(kt == 0), stop=(kt == CT - 1))
                    nc.scalar.activation(q_sb[:, n * 512:(n + 1) * 512],
                                         psq[:],
                                         AF.Identity, bias=qb[:], scale=1.0)
                    nc.scalar.activation(k_sb[:, n * 512:(n + 1) * 512],
                                         psk[:], AF.Copy)

                # ---- e^T matmuls + exp -> block-diag tiles (unscaled) ----
                kv = k_sb[:].rearrange("p (h w) -> p w h", w=W)
                qv = q_sb[:].rearrange("p (h w) -> p w h", w=W)
                for par in range(2):
                    for g in range(8):
                        ps = psp.tile([64, 256], dt.float32, tag="small",
                                      bufs=4)
                        for m in range(4):
                            w = 8 * g + 2 * m + par
                            nc.tensor.matmul(ps[:, m * 64:(m + 1) * 64],
                                             kv[:, w, :], qv[:, w, :],
                                             start=True, stop=True)
                        e16 = sp.tile([64, 256], dt.bfloat16, tag="e16", bufs=4)
                        nc.scalar.activation(e16[:], ps[:], AF.Exp)
                        nc.vector.tensor_mul(
                            blk_view(expblkH, par, g),
                            e16[:].rearrange("p (a b) -> p a b", a=4),
                            mask[:, None, :].broadcast_to((64, 4, 64)))
                for par in range(2):
                    for g in range(8):
                        ps = psp.tile([64, 256], dt.float32, tag="small",
                                      bufs=4)
                        for m in range(4):
                            h = 8 * g + 2 * m + par
                            nc.tensor.matmul(ps[:, m * 64:(m + 1) * 64],
                                             k_sb[:, h * W:(h + 1) * W],
                                             q_sb[:, h * W:(h + 1) * W],
                                             start=True, stop=True)
                        nc.scalar.activation(
                            blk_view(expblkW, par, g), ps[:], AF.Exp)

            with tc.tile_pool(name="cca_rows", bufs=1) as rp:
                rowC = rp.tile([1, 4096], dt.float32)
                rowR = rp.tile([1, 4096], dt.bfloat16)
                rowCb = rp.tile([1, 4096], dt.bfloat16)
                # denominators in pair-interleaved block order:
                # rowC[128t+64p+h] = sH(px h, 2t+p); rowR[128t+64p+w] = sW
                for n in range(8):
                    sl = slice(n * 512, (n + 1) * 512)
                    for row, blk in ((rowC, expblkH), (rowR, expblkW)):
                        ps = psp.tile([1, 512], dt.float32, tag="small",
                                      bufs=4)
                        nc.tensor.matmul(ps[:], og[:], blk[:, sl],
                                         start=True, stop=True)
                        nc.scalar.copy(row[:, sl], ps[:])

                # s = sH + sW: rowC[(w2,wp,h2,hp)] += rowR[(h2,hp,w2,wp)]
                for hp in range(2):
                    ov = rowC[:, hp::2].rearrange(
                        "p (w2 wp h2) -> p w2 wp h2", w2=32, wp=2)
                    iv = rowR[:].rearrange(
                        "p (h2 hp w2 wp) -> p hp w2 wp h2", h2=32, hp=2,
                        w2=32, wp=2)[:, hp]
                    nc.vector.tensor_add(ov, ov, iv)
                # recip in place -> rowC = r in (w2,wp,h) order
                nc.vector.reciprocal_approx_fast(out=rowC[:], in_=rowC[:])
                # rowR = r in (h2,hp,w) order
                for wp in range(2):
                    ov = rowR[:, wp::2].rearrange(
                        "p (h2 hp w2) -> p h2 hp w2", h2=32, hp=2)
                    iv = rowC[:].rearrange(
                        "p (w2 wp h2 hp) -> p wp h2 hp w2", w2=32, wp=2,
                        h2=32, hp=2)[:, wp]
                    nc.vector.tensor_copy(ov, iv)

                # ---- vproj dir W (rows): lhsT = row-pair chunks of frm ----
                for t in range(16):
                    ps = psp.tile([128, C], dt.float32, tag="big", bufs=4)
                    for kt in range(CT):
                        nc.tensor.matmul(
                            ps[:], frm[:, kt, 128 * t:128 * (t + 1)], vw[:, kt, :],
                            start=(kt == 0), stop=(kt == CT - 1))
                    v = vpool.tile([128, C], dt.bfloat16, tag=f"v{t}",
                                   name=f"vW{t}")
                    nc.scalar.activation(v[:], ps[:], AF.Copy)
                    vt[t] = v
                nc.vector.tensor_copy(rowCb[:], rowC[:])

                # broadcast via PE ones-matmul into PSUM, scale in place
                # (zeros stay zero); W first (aggW consumes it first)
                for bi, (blk, row) in enumerate(((expblkW, rowR),
                                                 (expblkH, rowCb))):
                    for n in range(8):
                        sl = slice(n * 512, (n + 1) * 512)
                        ps = psp.tile([128, 512], dt.float32, tag="small",
                                      bufs=4, name=f"bc{bi}_{n}")
                        nc.tensor.matmul(ps[:], ones1b[:], row[:, sl],
                                         start=True, stop=True)
                        nc.vector.tensor_mul(blk[:, sl], blk[:, sl], ps[:])

                # ---- vproj dir W, second half ----
                for t in range(16, 32):
                    ps = psp.tile([128, C], dt.float32, tag="big", bufs=4)
                    for kt in range(CT):
                        nc.tensor.matmul(
                            ps[:], frm[:, kt, 128 * t:128 * (t + 1)], vw[:, kt, :],
                            start=(kt == 0), stop=(kt == CT - 1))
                    v = vpool.tile([128, C], dt.bfloat16, tag=f"v{t}",
                                   name=f"vW{t}")
                    nc.scalar.activation(v[:], ps[:], AF.Copy)
                    vt[t] = v
                vtW = list(vt)


        with tc.tile_pool(name="cca_fcm", bufs=1) as fcmp:
            fcm = fcmp.tile([128, CT, HW], dt.float16)
            for ct in range(CT):
                nc.vector.tensor_copy(
                    fcm[:, ct, :].rearrange("p (w h) -> p w h", w=W),
                    _interior(fin, ct).rearrange("p h w -> p w h"))

            # ---- aggregation dir W: psum [c, 8 rows, 64] -> fout ----
            for grp in range(8):
                for cc in range(CT):
                    ps = psp.tile([128, 512], dt.float32, tag="big", bufs=4)
                    for tt in range(4):
                        t = grp * 4 + tt
                        nc.tensor.matmul(
                            ps[:, tt * 128:(tt + 1) * 128],
                            vtW[t][:, cc * 128:(cc + 1) * 128],
                            expblkW[:, t * 128:(t + 1) * 128],
                            start=True, stop=True)
                    rows = _pad_hw(fout, cc)[:, 1 + grp * 8:1 + grp * 8 + 8,
                                             1:1 + W]
                    finr = _pad_hw(fin, cc)[:, 1 + grp * 8:1 + grp * 8 + 8,
                                            1:1 + W]
                    nc.vector.scalar_tensor_tensor(
                        out=rows,
                        in0=ps[:].rearrange("p (h w) -> p h w", w=W),
                        scalar=gvb[:, cc:cc + 1], in1=finr,
                        op0=ALU.add, op1=ALU.add)

            # ---- vproj dir H (cols): lhsT = col-pair chunks of fcm ----
            for t in range(32):
                ps = psp.tile([128, C], dt.float32, tag="big", bufs=4)
                for kt in range(CT):
                    nc.tensor.matmul(
                        ps[:], fcm[:, kt, 128 * t:128 * (t + 1)], vw[:, kt, :],
                        start=(kt == 0), stop=(kt == CT - 1))
                v = vpool.tile([128, C], dt.bfloat16, tag=f"v{t}",
                               name=f"vH{t}")
                nc.scalar.activation(v[:], ps[:], AF.Copy)
                vt[t] = v
            vtH = list(vt)

        with tc.tile_pool(name="cca_fc", bufs=1) as fcp:
            foutC = fcp.tile([128, CT, HW], dt.float16)
            # ---- aggregation dir H -> foutC (col-major), merge per cc ----
            for cc in range(CT):
                for grp in range(8):
                    ps = psp.tile([128, 512], dt.float32, tag="big", bufs=4)
                    for tt in range(4):
                        t = grp * 4 + tt
                        nc.tensor.matmul(
                            ps[:, tt * 128:(tt + 1) * 128],
                            vtH[t][:, cc * 128:(cc + 1) * 128],
                            expblkH[:, t * 128:(t + 1) * 128],
                            start=True, stop=True)
                    nc.scalar.activation(
                        foutC[:, cc, grp * 512:(grp + 1) * 512], ps[:],
                        AF.Copy)
                iv = _interior(fout, cc)
                nc.vector.tensor_add(
                    iv, iv,
                    foutC[:, cc, :].rearrange("p (w h) -> p h w", h=H))


# ---------------- host side ----------------

def _prep_inputs(x, conv1_w, bn1_g, bn1_b, bn1_m, bn1_v,
                 q_w, q_b, k_w, k_b, v_w, v_b, cca_gamma,
                 conv2_w, bn2_g, bn2_b, bn2_m, bn2_v):
    f16 = np.float16
    eps = 1e-5
    gamma = float(cca_gamma)
    x = np.asarray(x)
    conv1_w = np.asarray(conv1_w)
    conv2_w = np.asarray(conv2_w)
    assert np.max(np.abs(np.asarray(k_b))) == 0.0, "nonzero k bias unsupported"

    bn1_scale = (np.asarray(bn1_g) / np.sqrt(np.asarray(bn1_v) + eps)).astype(np.float32)
    bn1_shift = (np.asarray(bn1_b) - np.asarray(bn1_m) * bn1_scale).astype(np.float32)
    bn1_t = np.ascontiguousarray(
        np.stack([bn1_scale.reshape(CT, 128).T, bn1_shift.reshape(CT, 128).T],
                 axis=-1), np.float32)

    qk_t = np.concatenate([np.asarray(q_w).T, np.asarray(k_w).T], axis=1)
    qkw_t = np.ascontiguousarray(qk_t.reshape(CT, 128, 128), f16)
    qb_t = np.asarray(q_b).reshape(64, 1).astype(np.float32)
    vw_t = np.ascontiguousarray(np.asarray(v_w).T.reshape(CT, 128, C), f16)
    gvb_t = np.ascontiguousarray((gamma * np.asarray(v_b)).reshape(CT, 128).T,
                                 np.float32)
    og_t = np.full((128, 1), 1.0 / gamma, ml_dtypes.bfloat16)
    mask_t = np.ascontiguousarray((1.0 - np.eye(64)).astype(ml_dtypes.bfloat16))

    bn2_scale = (np.asarray(bn2_g) / np.sqrt(np.asarray(bn2_v) + eps)).astype(np.float32)
    bn2_shift = (np.asarray(bn2_b) - np.asarray(bn2_m) * bn2_scale).astype(np.float32)

    common = dict(qkw=qkw_t, qb=qb_t, vw=vw_t, gvb=gvb_t, og=og_t,
                  mask=mask_t, bn1=bn1_t)

    in_maps = []
    for core in range(N_CORES):
        b, half = core // 2, core % 2
        xs = x[b, half * 1024:(half + 1) * 1024].reshape(KT1, 128, HW).astype(f16)
        w1s = conv1_w[:, half * 1024:(half + 1) * 1024]
        w1s = w1s.reshape(C, KT1, 128, 3, 3).transpose(1, 3, 4, 2, 0) \
            .reshape(KT1, 9, 128, CT, 128).transpose(0, 3, 1, 2, 4) \
            .reshape(KT1, CT, 9, 128, 128).astype(f16)
        w2s = conv2_w[half * 256:(half + 1) * 256]
        w2s = w2s.reshape(256, CT, 128, 3, 3).transpose(1, 3, 4, 2, 0) \
            .reshape(CT, 9, 128, 256).astype(f16)
        bs = bn2_scale[half * 256:(half + 1) * 256].reshape(2, 128).T
        bh = bn2_shift[half * 256:(half + 1) * 256].reshape(2, 128).T
        bn2_t = np.ascontiguousarray(np.stack([bs, bh], axis=-1), np.float32)
        in_maps.append(dict(common, x=np.ascontiguousarray(xs),
                            w1=np.ascontiguousarray(w1s),
                            w2=np.ascontiguousarray(w2s), bn2=bn2_t))
    return in_maps


def _get_compiled(debug_stage):
    key = debug_stage
    if key not in _COMPILED:
        _COMPILED[key] = build_kernel(debug_stage)
    return _COMPILED[key]


def run(inputs, debug_stage=None, trace=False, **kwargs):
    nc = _get_compiled(debug_stage)
    in_maps = _prep_inputs(**inputs)
    return run_bass_kernel_spmd(nc, in_maps, list(range(N_CORES)), trace=trace,
                                **kwargs)


def kernel(**inputs):
    res = run(inputs, debug_stage=DEBUG_STAGE)
    out = np.empty((B, C, H, W), np.float32)
    if DEBUG_STAGE is None:
        for core in range(N_CORES):
            b, half = core // 2, core % 2
            out[b, half * 256:(half + 1) * 256] = \
                res.results[core]["out"].reshape(256, H, W)
    else:
        for b in range(B):
            out[b] = res.results[2 * b]["out"].reshape(C, H, W)
    return out


# revision 30
# speedup vs baseline: 1.0044x; 1.0044x over previous
"""CCNet unit (conv3x3 -> BN/ReLU -> 2x criss-cross attention -> conv3x3 ->
BN/ReLU) on 8 Trainium2 NeuronCores.

Sharding (SPMD-symmetric program; per-core differences live only in data):
  core = 2*b + half   (b = sample 0..3, half = 0/1)
  - conv1 (Cin=2048): input-channel split across the pair; partial sums
    pair-AllReduce'd (fp16, in h-halves), then BN+ReLU on both cores.
  - CCA x2: computed redundantly by both pair members (cheap vs conv1).
  - conv2 (Cout=512): output-channel split via sharded weights.

All matmuls fp16 (full PE rate), PSUM accumulates fp32.

Attention layouts (parity-packed so matmul operand base partitions match):
  expT_e/o  [64, 2048] : exp(eH^T) at [i, (w//2)*64 + h] for even/odd w,
                         diag-masked, unnormalized
  expTW_e/o [64, 2048] : exp(eW^T) at [j, (h//2)*64 + w] for even/odd h
  expblkH   [128, 4096]: block-diag normalized exp for column pairs
                         (2t,2t+1): [0:64, 128t:128t+64] = even,
                         [64:128, 128t+64:128t+128] = odd, rest zero
  expblkW   [128, 4096]: same for row pairs
  vtW[t]    [128, 512] : v rows (2t,2t+1): partition 64*(h%2)+w
  vtH[t]    [128, 512] : v cols (2t,2t+1): partition 64*(w%2)+h
Aggregation matmuls consume (vt, expblk) pairs directly: [128,128] rhs
per pair -> psum [c, 8 rows/cols, 64], drained contiguously.
Softmax has no max-subtraction (logits bounded ~ +-5); normalization
(gamma/s) is folded into expblk; v-bias*gamma (gvb) added at the dir-W
drain; dir-H lands in a compact col-major buffer merged by one
transposed add per channel tile.
"""

import sys

sys.path.insert(0, "/opt/trn_rl_repo")

import numpy as np
import ml_dtypes

import concourse.bacc as bacc
import concourse.mybir as mybir
import concourse.tile as tile
from concourse.bass_utils import run_bass_kernel_spmd

dt = mybir.dt
AF = mybir.ActivationFunctionType
ALU = mybir.AluOpType

B, CIN, C, CR, H, W = 4, 2048, 512, 64, 64, 64
HW = H * W
WP = W + 2
PADPX = (H + 2) * WP
INT0 = WP + 1
KT1 = CIN // 2 // 128   # 8
CT = C // 128           # 4
N_CORES = 8
PAIRS = [[0, 1], [2, 3], [4, 5], [6, 7]]

DEBUG_STAGE = None
_COMPILED = {}


def _pad_hw(t, ct=None):
    """[128, 66, 66] padded view (or of channel-tile ct)."""
    v = t[:, ct, :] if ct is not None else t[:]
    return v.rearrange("p (h w) -> p h w", w=WP)


def _interior(t, ct=None):
    """[128, 64, 64] interior view."""
    return _pad_hw(t, ct)[:, 1:1 + H, 1:1 + W]


def _chunk_rhs(t, ct, h0, off):
    """[128, 8, 64] rhs AP: 8 output rows from h0, reading offset `off`."""
    r0, c0 = h0 + off // WP, off % WP
    v = _pad_hw(t, ct)
    return v[:, r0:r0 + 8, c0:c0 + 64]


def _memset_border(nc, t, ct=None, eng=None):
    """Zero only the padding border of a [*, (66,66)] padded tile."""
    eng = eng or nc.vector
    v = _pad_hw(t, ct)
    eng.memset(v[:, 0, :], 0.0)
    eng.memset(v[:, H + 1, :], 0.0)
    # (row r, col 65) and (row r+1, col 0) pairs, r = 0..64
    base = t[:, ct, :] if ct is not None else t[:]
    strip = base[:, W + 1:W + 1 + 65 * WP].rearrange(
        "p (r c) -> p r c", c=WP)[:, :, 0:2]
    eng.memset(strip, 0.0)


def build_kernel(debug_stage=None):
    nc = bacc.Bacc("TRN2", target_bir_lowering=False, debug=False,
                   num_devices=N_CORES)

    x_in = nc.dram_tensor("x", [KT1, 128, HW], dt.float16, kind="ExternalInput").ap()
    w1_in = nc.dram_tensor("w1", [KT1, CT, 9, 128, 128], dt.float16, kind="ExternalInput").ap()
    bn1_in = nc.dram_tensor("bn1", [128, CT, 2], dt.float32, kind="ExternalInput").ap()
    qkw_in = nc.dram_tensor("qkw", [CT, 128, 128], dt.float16, kind="ExternalInput").ap()
    qb_in = nc.dram_tensor("qb", [64, 1], dt.float32, kind="ExternalInput").ap()
    vw_in = nc.dram_tensor("vw", [CT, 128, C], dt.float16, kind="ExternalInput").ap()
    gvb_in = nc.dram_tensor("gvb", [128, CT], dt.float32, kind="ExternalInput").ap()
    og_in = nc.dram_tensor("og", [128, 1], dt.bfloat16, kind="ExternalInput").ap()
    mask_in = nc.dram_tensor("mask", [64, 64], dt.bfloat16, kind="ExternalInput").ap()
    w2_in = nc.dram_tensor("w2", [CT, 9, 128, 256], dt.float16, kind="ExternalInput").ap()
    bn2_in = nc.dram_tensor("bn2", [128, 2, 2], dt.float32, kind="ExternalInput").ap()

    if debug_stage is None:
        out_t = nc.dram_tensor("out", [2, 128, HW], dt.float32, kind="ExternalOutput").ap()
    else:
        out_t = nc.dram_tensor("out", [CT, 128, HW], dt.float32, kind="ExternalOutput").ap()

    with tile.TileContext(nc) as tc:
        _emit(nc, tc, debug_stage, x_in, w1_in, bn1_in, qkw_in, qb_in, vw_in,
              gvb_in, og_in, mask_in, w2_in, bn2_in, out_t)
    nc.compile()
    return nc


def _emit(nc, tc, debug_stage, x_in, w1_in, bn1_in, qkw_in, qb_in, vw_in,
          gvb_in, og_in, mask_in, w2_in, bn2_in, out_t):
    from contextlib import ExitStack

    ctx = ExitStack()
    with ctx:
        pool_feats = ctx.enter_context(tc.tile_pool(name="feats", bufs=1))
        featsA = pool_feats.tile([128, CT, PADPX], dt.float16)
        for ct in range(CT):
            _memset_border(nc, featsA, ct,
                           eng=nc.vector if ct % 2 == 0 else nc.gpsimd)

        pool_const = ctx.enter_context(tc.tile_pool(name="const", bufs=1))
        bn1 = pool_const.tile([128, CT, 2], dt.float32)
        qkw = pool_const.tile([128, CT, 128], dt.float16)
        qb = pool_const.tile([64, 1], dt.float32)
        vw = pool_const.tile([128, CT, C], dt.float16)
        gvb = pool_const.tile([128, CT], dt.float32)
        og = pool_const.tile([128, 1], dt.bfloat16)
        mask = pool_const.tile([64, 64], dt.bfloat16)
        ones1b = pool_const.tile([1, 128], dt.bfloat16)
        nc.vector.memset(ones1b[:], 1.0)
        nc.sync.dma_start(bn1[:], bn1_in[:])
        nc.sync.dma_start(qkw[:], qkw_in[:].rearrange("k p c -> p k c"))
        nc.sync.dma_start(qb[:], qb_in[:])
        nc.sync.dma_start(vw[:], vw_in[:].rearrange("k p c -> p k c"))
        nc.sync.dma_start(gvb[:], gvb_in[:])
        nc.sync.dma_start(og[:], og_in[:])
        nc.sync.dma_start(mask[:], mask_in[:])

        # ---------------- conv1 ----------------
        with (
            tc.tile_pool(name="c1x", bufs=1) as c1x,
            tc.tile_pool(name="c1w", bufs=1) as c1w,
            tc.tile_pool(name="c1ps", bufs=1, space="PSUM") as c1ps,
            tc.tile_pool(name="c1st", bufs=4) as c1st,
            tc.tile_pool(name="c1dram", bufs=1, space="DRAM") as c1dram,
        ):
            xts, wts = [], []
            for kt in range(KT1):
                wt = c1w.tile([128, 9, C], dt.float16, tag=f"w{kt}")
                xt = c1x.tile([128, PADPX], dt.float16, tag=f"x{kt}")
                _memset_border(nc, xt,
                               eng=nc.vector if kt % 2 == 0 else nc.gpsimd)
                xv = x_in[kt].rearrange("p (h w) -> p h w", w=W)
                iv = _interior(xt)
                if kt == 0:
                    # first chunks small so mt0/kt0 compute starts early
                    nc.sync.dma_start(wt[:, :, 0:128],
                                      w1_in[kt, 0].rearrange("t p c -> p t c"))
                    for rc in range(4):
                        nc.sync.dma_start(iv[:, rc * 16:(rc + 1) * 16, :],
                                          xv[:, rc * 16:(rc + 1) * 16, :])
                else:
                    for mt in range(CT):
                        nc.sync.dma_start(
                            wt[:, :, mt * 128:(mt + 1) * 128],
                            w1_in[kt, mt].rearrange("t p c -> p t c"))
                    nc.sync.dma_start(iv, xv)
                xts.append(xt)
                wts.append(wt)
                if kt == 1:
                    for mt in range(1, CT):
                        nc.sync.dma_start(
                            wts[0][:, :, mt * 128:(mt + 1) * 128],
                            w1_in[0, mt].rearrange("t p c -> p t c"))

            partial = c1dram.tile([CT, 4, 128, HW // 4], dt.float16)
            reduced = c1dram.tile([CT, 4, 128, HW // 4], dt.float16)

            def drain(mt, h8):
                st = c1st.tile([128, 512], dt.float16, tag="st",
                               name=f"st{mt}_{h8}")
                nc.scalar.activation(st[:], pss[h8][:], AF.Copy)
                nc.sync.dma_start(
                    partial[mt, h8 // 2, :, (h8 % 2) * 512:(h8 % 2 + 1) * 512],
                    st[:])

            def allreduce(mt, q0=0, q1=4):
                if q1 - q0 == 4:
                    nc.gpsimd.collective_compute(
                        "AllReduce", ALU.add, replica_groups=PAIRS,
                        ins=[partial[mt]], outs=[reduced[mt]])
                else:
                    nc.gpsimd.collective_compute(
                        "AllReduce", ALU.add, replica_groups=PAIRS,
                        ins=[partial[mt, q0:q1]], outs=[reduced[mt, q0:q1]])
                for h8 in range(q0 * 2, q1 * 2):
                    red_sb = c1st.tile([128, 512], dt.float16, tag="redsb",
                                       name=f"red{mt}_{h8}")
                    nc.sync.dma_start(
                        red_sb[:],
                        reduced[mt, h8 // 2, :,
                                (h8 % 2) * 512:(h8 % 2 + 1) * 512])
                    nc.scalar.activation(
                        _pad_hw(featsA, mt)[:, 1 + h8 * 8:1 + h8 * 8 + 8,
                                            1:1 + W],
                        red_sb[:].rearrange("p (h w) -> p h w", w=W),
                        AF.Relu, bias=bn1[:, mt, 1:2], scale=bn1[:, mt, 0:1])

            # mt=0: kt-outer so compute starts as soon as x[0]/w[0] land
            pss = [c1ps.tile([128, 512], dt.float32, tag=f"ps{h8}",
                             name=f"ps0_{h8}") for h8 in range(8)]
            for kt in range(KT1):
                for h8 in range(8):
                    for tap in range(9):
                        dy, dx = tap // 3, tap % 3
                        nc.tensor.matmul(
                            pss[h8][:],
                            wts[kt][:, tap, 0:128],
                            _pad_hw(xts[kt])[:, h8 * 8 + dy:h8 * 8 + dy + 8,
                                             dx:dx + 64],
                            start=(kt == 0 and tap == 0),
                            stop=(kt == KT1 - 1 and tap == 8))
            for h8 in range(8):
                drain(0, h8)
            allreduce(0)

            # mt=1..3: h0-outer (streaming drains, early AllReduce issue)
            for mt in range(1, CT):
                pss = [c1ps.tile([128, 512], dt.float32, tag=f"ps{h8}",
                                 name=f"ps{mt}_{h8}") for h8 in range(8)]
                for h8 in range(8):
                    i = 0
                    for kt in range(KT1):
                        for tap in range(9):
                            dy, dx = tap // 3, tap % 3
                            nc.tensor.matmul(
                                pss[h8][:],
                                wts[kt][:, tap, mt * 128:(mt + 1) * 128],
                                _pad_hw(xts[kt])[:, h8 * 8 + dy:h8 * 8 + dy + 8,
                                                 dx:dx + 64],
                                start=(i == 0), stop=(i == KT1 * 9 - 1))
                            i += 1
                    drain(mt, h8)
                    if mt == CT - 1 and h8 == 3:
                        allreduce(mt, 0, 2)   # overlaps h8=4..7 compute
                    if mt == CT - 1 and h8 == 5:
                        allreduce(mt, 2, 3)   # overlaps h8=6..7 compute
                if mt < CT - 1:
                    allreduce(mt)
                else:
                    allreduce(mt, 3, 4)

        if debug_stage == "feats1":
            _emit_debug_out(nc, tc, featsA, out_t)
            return

        # ---------------- CCA x2 ----------------
        pool_fb = ctx.enter_context(tc.tile_pool(name="featsB", bufs=1))
        featsB = pool_fb.tile([128, CT, PADPX], dt.float16)
        pool_blk = ctx.enter_context(tc.tile_pool(name="expblk", bufs=1))
        expblkH = pool_blk.tile([128, 4096], dt.bfloat16)
        expblkW = pool_blk.tile([128, 4096], dt.bfloat16)
        nc.vector.memset(expblkH[:], 0.0)
        nc.gpsimd.memset(expblkW[:], 0.0)
        for ct in range(CT):
            _memset_border(nc, featsB, ct,
                           eng=nc.vector if ct % 2 == 0 else nc.gpsimd)
        _emit_cca(nc, tc, featsA, featsB, qkw, qb, vw, gvb, og, mask,
                  ones1b, expblkH, expblkW)
        if debug_stage == "cca1":
            _emit_debug_out(nc, tc, featsB, out_t)
            return
        pool_c2 = ctx.enter_context(tc.tile_pool(name="c2w", bufs=1))
        w2 = pool_c2.tile([128, CT, 9, 256], dt.float16)
        bn2 = pool_c2.tile([128, 2, 2], dt.float32)
        nc.sync.dma_start(bn2[:], bn2_in[:])
        for kt in range(CT):
            nc.sync.dma_start(w2[:, kt, :, :],
                              w2_in[kt].rearrange("t p c -> p t c"))

        _emit_cca(nc, tc, featsB, featsA, qkw, qb, vw, gvb, og, mask,
                  ones1b, expblkH, expblkW)
        if debug_stage == "cca2":
            _emit_debug_out(nc, tc, featsA, out_t)
            return

        # ---------------- conv2 ----------------
        with (
            tc.tile_pool(name="c2ps", bufs=8, space="PSUM") as c2ps,
            tc.tile_pool(name="c2st", bufs=4) as c2st,
        ):
            for mt in range(2):
                for h0 in range(0, H, 8):
                    ps = c2ps.tile([128, 512], dt.float32)
                    i = 0
                    for kt in range(CT):
                        for dy in range(3):
                            for dx in range(3):
                                nc.tensor.matmul(
                                    ps[:],
                                    w2[:, kt, dy * 3 + dx, mt * 128:(mt + 1) * 128],
                                    _chunk_rhs(featsA, kt, h0, dy * WP + dx),
                                    start=(i == 0), stop=(i == CT * 9 - 1))
                                i += 1
                    st = c2st.tile([128, 512], dt.float32)
                    nc.scalar.activation(st[:], ps[:], AF.Relu,
                                         bias=bn2[:, mt, 1:2],
                                         scale=bn2[:, mt, 0:1])
                    nc.sync.dma_start(out_t[mt, :, h0 * W:(h0 + 8) * W], st[:])


def _emit_debug_out(nc, tc, feats, out_t):
    with tc.tile_pool(name="dbg", bufs=4) as dbg:
        for ct in range(CT):
            st = dbg.tile([128, HW], dt.float32)
            nc.vector.tensor_copy(
                st[:].rearrange("p (h w) -> p h w", w=W), _interior(feats, ct))
            nc.sync.dma_start(out_t[ct], st[:])


def _emit_cca(nc, tc, fin, fout, qkw, qb, vw, gvb, og, mask, ones1b,
              expblkH, expblkW):
    """fout = gamma*cca(fin) + fin (interior; fout border must be zero)."""
    from contextlib import ExitStack

    ctx = ExitStack()
    with ctx:
        sb = ctx.enter_context(tc.tile_pool(name="cca_sb", bufs=1))
        vpool = ctx.enter_context(tc.tile_pool(name="cca_v", bufs=1))
        psp = ctx.enter_context(tc.tile_pool(name="cca_ps", bufs=1, space="PSUM"))

        def blk_view(blk, par, g):
            v = blk[64 * par:64 * (par + 1), :].rearrange(
                "p (t x) -> p t x", x=128)
            return v[:, 4 * g:4 * (g + 1), 64 * par:64 * (par + 1)]

        vt = [None] * 32
        with tc.tile_pool(name="cca_frm", bufs=1) as frmp:
            frm = frmp.tile([128, CT, HW], dt.float16)
            with (
                tc.tile_pool(name="cca_qk", bufs=1) as qkp,
                tc.tile_pool(name="cca_small", bufs=2) as sp,
            ):
                for ct in range(CT):
                    nc.sync.dma_start(
                        frm[:, ct, :].rearrange("p (h w) -> p h w", w=W),
                        _interior(fin, ct))

                # ---- q, k projections (separate half-width matmuls so
                # both drain at partition base 0; no DMA shuffles) ----
                q_sb = qkp.tile([64, HW], dt.float16)
                k_sb = qkp.tile([64, HW], dt.float16)
                for n in range(8):
                    psq = psp.tile([64, 512], dt.float32, tag="small",
                                   bufs=4, name=f"psq{n}")
                    psk = psp.tile([64, 512], dt.float32, tag="small",
                                   bufs=4, name=f"psk{n}")
                    for kt in range(CT):
                        rhs = _chunk_rhs(fin, kt, n * 8, INT0)
                        nc.tensor.matmul(psq[:], qkw[:, kt, 0:64], rhs,
                                         start=(kt == 0), stop=(kt == CT - 1))
                        nc.tensor.matmul(psk[:], qkw[:, kt, 64:128], rhs,
                                         start=(kt == 0), stop=(kt == CT - 1))
                    nc.scalar.activation(q_sb[:, n * 512:(n + 1) * 512],
                                         psq[:],
                                         AF.Identity, bias=qb[:], scale=1.0)
                    nc.scalar.activation(k_sb[:, n * 512:(n + 1) * 512],
                                         psk[:], AF.Copy)

                # ---- e^T matmuls + exp -> block-diag tiles (unscaled) ----
                kv = k_sb[:].rearrange("p (h w) -> p w h", w=W)
                qv = q_sb[:].rearrange("p (h w) -> p w h", w=W)
                for par in range(2):
                    for g in range(8):
                        ps = psp.tile([64, 256], dt.float32, tag="small",
                                      bufs=4)
                        for m in range(4):
                            w = 8 * g + 2 * m + par
                            nc.tensor.matmul(ps[:, m * 64:(m + 1) * 64],
                                             kv[:, w, :], qv[:, w, :],
                                             start=True, stop=True)
                        e16 = sp.tile([64, 256], dt.bfloat16, tag="e16", bufs=4)
                        nc.scalar.activation(e16[:], ps[:], AF.Exp)
                        nc.vector.tensor_mul(
                            blk_view(expblkH, par, g),
                            e16[:].rearrange("p (a b) -> p a b", a=4),
                            mask[:, None, :].broadcast_to((64, 4, 64)))
                for par in range(2):
                    for g in range(8):
                        ps = psp.tile([64, 256], dt.float32, tag="small",
                                      bufs=4)
                        for m in range(4):
                            h = 8 * g + 2 * m + par
                            nc.tensor.matmul(ps[:, m * 64:(m + 1) * 64],
                                             k_sb[:, h * W:(h + 1) * W],
                                             q_sb[:, h * W:(h + 1) * W],
                                             start=True, stop=True)
                        nc.scalar.activation(
                            blk_view(expblkW, par, g), ps[:], AF.Exp)

            with tc.tile_pool(name="cca_rows", bufs=1) as rp:
                rowC = rp.tile([1, 4096], dt.float32)
                rowR = rp.tile([1, 4096], dt.bfloat16)
                rowCb = rp.tile([1, 4096], dt.bfloat16)
                # denominators in pair-interleaved block order:
                # rowC[128t+64p+h] = sH(px h, 2t+p); rowR[128t+64p+w] = sW
                for n in range(8):
                    sl = slice(n * 512, (n + 1) * 512)
                    for row, blk in ((rowC, expblkH), (rowR, expblkW)):
                        ps = psp.tile([1, 512], dt.float32, tag="small",
                                      bufs=4)
                        nc.tensor.matmul(ps[:], og[:], blk[:, sl],
                                         start=True, stop=True)
                        nc.scalar.copy(row[:, sl], ps[:])

                # s = sH + sW: rowC[(w2,wp,h2,hp)] += rowR[(h2,hp,w2,wp)]
                for hp in range(2):
                    ov = rowC[:, hp::2].rearrange(
                        "p (w2 wp h2) -> p w2 wp h2", w2=32, wp=2)
                    iv = rowR[:].rearrange(
                        "p (h2 hp w2 wp) -> p hp w2 wp h2", h2=32, hp=2,
                        w2=32, wp=2)[:, hp]
                    nc.vector.tensor_add(ov, ov, iv)
                # recip in place -> rowC = r in (w2,wp,h) order
                nc.vector.reciprocal_approx_fast(out=rowC[:], in_=rowC[:])
                # rowR = r in (h2,hp,w) order
                for wp in range(2):
                    ov = rowR[:, wp::2].rearrange(
                        "p (h2 hp w2) -> p h2 hp w2", h2=32, hp=2)
                    iv = rowC[:].rearrange(
                        "p (w2 wp h2 hp) -> p wp h2 hp w2", w2=32, wp=2,
                        h2=32, hp=2)[:, wp]
                    nc.vector.tensor_copy(ov, iv)

                # ---- vproj dir W (rows): lhsT = row-pair chunks of frm ----
                for t in range(16):
                    ps = psp.tile([128, C], dt.float32, tag="big", bufs=4)
                    for kt in range(CT):
                        nc.tensor.matmul(
                            ps[:], frm[:, kt, 128 * t:128 * (t + 1)], vw[:, kt, :],
                            start=(kt == 0), stop=(kt == CT - 1))
                    v = vpool.tile([128, C], dt.bfloat16, tag=f"v{t}",
                                   name=f"vW{t}")
                    nc.scalar.activation(v[:], ps[:], AF.Copy)
                    vt[t] = v
                nc.vector.tensor_copy(rowCb[:], rowC[:])

                # broadcast via PE ones-matmul into PSUM, scale in place
                # (zeros stay zero); W first (aggW consumes it first)
                for bi, (blk, row) in enumerate(((expblkW, rowR),
                                                 (expblkH, rowCb))):
                    for n in range(8):
                        sl = slice(n * 512, (n + 1) * 512)
                        ps = psp.tile([128, 512], dt.float32, tag="small",
                                      bufs=4, name=f"bc{bi}_{n}")
                        nc.tensor.matmul(ps[:], ones1b[:], row[:, sl],
                                         start=True, stop=True)
                        nc.vector.tensor_mul(blk[:, sl], blk[:, sl], ps[:])

                # ---- vproj dir W, second half ----
                for t in range(16, 32):
                    ps = psp.tile([128, C], dt.float32, tag="big", bufs=4)
                    for kt in range(CT):
                        nc.tensor.matmul(
                            ps[:], frm[:, kt, 128 * t:128 * (t + 1)], vw[:, kt, :],
                            start=(kt == 0), stop=(kt == CT - 1))
                    v = vpool.tile([128, C], dt.bfloat16, tag=f"v{t}",
                                   name=f"vW{t}")
                    nc.scalar.activation(v[:], ps[:], AF.Copy)
                    vt[t] = v
                vtW = list(vt)


        with tc.tile_pool(name="cca_fcm", bufs=1) as fcmp:
            fcm = fcmp.tile([128, CT, HW], dt.float16)
            # ---- aggregation dir W: psum [c, 8 rows, 64] -> fout ----
            for grp in range(8):
                for cc in range(CT):
                    ps = psp.tile([128, 512], dt.float32, tag="big", bufs=4)
                    for tt in range(4):
                        t = grp * 4 + tt
                        nc.tensor.matmul(
                            ps[:, tt * 128:(tt + 1) * 128],
                            vtW[t][:, cc * 128:(cc + 1) * 128],
                            expblkW[:, t * 128:(t + 1) * 128],
                            start=True, stop=True)
                    rows = _pad_hw(fout, cc)[:, 1 + grp * 8:1 + grp * 8 + 8,
                                             1:1 + W]
                    finr = _pad_hw(fin, cc)[:, 1 + grp * 8:1 + grp * 8 + 8,
                                            1:1 + W]
                    nc.vector.scalar_tensor_tensor(
                        out=rows,
                        in0=ps[:].rearrange("p (h w) -> p h w", w=W),
                        scalar=gvb[:, cc:cc + 1], in1=finr,
                        op0=ALU.add, op1=ALU.add)

            for ct in range(CT):
                nc.vector.tensor_copy(
                    fcm[:, ct, :].rearrange("p (w h) -> p w h", w=W),
                    _interior(fin, ct).rearrange("p h w -> p w h"))

            # ---- vproj dir H (cols): lhsT = col-pair chunks of fcm ----
            for t in range(32):
                ps = psp.tile([128, C], dt.float32, tag="big", bufs=4)
                for kt in range(CT):
                    nc.tensor.matmul(
                        ps[:], fcm[:, kt, 128 * t:128 * (t + 1)], vw[:, kt, :],
                        start=(kt == 0), stop=(kt == CT - 1))
                v = vpool.tile([128, C], dt.bfloat16, tag=f"v{t}",
                               name=f"vH{t}")
                nc.scalar.activation(v[:], ps[:], AF.Copy)
                vt[t] = v
            vtH = list(vt)

        with tc.tile_pool(name="cca_fc", bufs=1) as fcp:
            foutC = fcp.tile([128, CT, HW], dt.float16)
            # ---- aggregation dir H -> foutC (col-major), merge per cc ----
            for cc in range(CT):
                for grp in range(8):
                    ps = psp.tile([128, 512], dt.float32, tag="big", bufs=4)
                    for tt in range(4):
                        t = grp * 4 + tt
                        nc.tensor.matmul(
                            ps[:, tt * 128:(tt + 1) * 128],
                            vtH[t][:, cc * 128:(cc + 1) * 128],
                            expblkH[:, t * 128:(t + 1) * 128],
                            start=True, stop=True)
                    nc.scalar.activation(
                        foutC[:, cc, grp * 512:(grp + 1) * 512], ps[:],
                        AF.Copy)
                iv = _interior(fout, cc)
                nc.vector.tensor_add(
                    iv, iv,
                    foutC[:, cc, :].rearrange("p (w h) -> p h w", h=H))


# ---------------- host side ----------------

def _prep_inputs(x, conv1_w, bn1_g, bn1_b, bn1_m, bn1_v,
                 q_w, q_b, k_w, k_b, v_w, v_b, cca_gamma,
                 conv2_w, bn2_g, bn2_b, bn2_m, bn2_v):
    f16 = np.float16
    eps = 1e-5
    gamma = float(cca_gamma)
    x = np.asarray(x)
    conv1_w = np.asarray(conv1_w)
    conv2_w = np.asarray(conv2_w)
    assert np.max(np.abs(np.asarray(k_b))) == 0.0, "nonzero k bias unsupported"

    bn1_scale = (np.asarray(bn1_g) / np.sqrt(np.asarray(bn1_v) + eps)).astype(np.float32)
    bn1_shift = (np.asarray(bn1_b) - np.asarray(bn1_m) * bn1_scale).astype(np.float32)
    bn1_t = np.ascontiguousarray(
        np.stack([bn1_scale.reshape(CT, 128).T, bn1_shift.reshape(CT, 128).T],
                 axis=-1), np.float32)

    qk_t = np.concatenate([np.asarray(q_w).T, np.asarray(k_w).T], axis=1)
    qkw_t = np.ascontiguousarray(qk_t.reshape(CT, 128, 128), f16)
    qb_t = np.asarray(q_b).reshape(64, 1).astype(np.float32)
    vw_t = np.ascontiguousarray(np.asarray(v_w).T.reshape(CT, 128, C), f16)
    gvb_t = np.ascontiguousarray((gamma * np.asarray(v_b)).reshape(CT, 128).T,
                                 np.float32)
    og_t = np.full((128, 1), 1.0 / gamma, ml_dtypes.bfloat16)
    mask_t = np.ascontiguousarray((1.0 - np.eye(64)).astype(ml_dtypes.bfloat16))

    bn2_scale = (np.asarray(bn2_g) / np.sqrt(np.asarray(bn2_v) + eps)).astype(np.float32)
    bn2_shift = (np.asarray(bn2_b) - np.asarray(bn2_m) * bn2_scale).astype(np.float32)

    common = dict(qkw=qkw_t, qb=qb_t, vw=vw_t, gvb=gvb_t, og=og_t,
                  mask=mask_t, bn1=bn1_t)

    in_maps = []
    for core in range(N_CORES):
        b, half = core // 2, core % 2
        xs = x[b, half * 1024:(half + 1) * 1024].reshape(KT1, 128, HW).astype(f16)
        w1s = conv1_w[:, half * 1024:(half + 1) * 1024]
        w1s = w1s.reshape(C, KT1, 128, 3, 3).transpose(1, 3, 4, 2, 0) \
            .reshape(KT1, 9, 128, CT, 128).transpose(0, 3, 1, 2, 4) \
            .reshape(KT1, CT, 9, 128, 128).astype(f16)
        w2s = conv2_w[half * 256:(half + 1) * 256]
        w2s = w2s.reshape(256, CT, 128, 3, 3).transpose(1, 3, 4, 2, 0) \
            .reshape(CT, 9, 128, 256).astype(f16)
        bs = bn2_scale[half * 256:(half + 1) * 256].reshape(2, 128).T
        bh = bn2_shift[half * 256:(half + 1) * 256].reshape(2, 128).T
        bn2_t = np.ascontiguousarray(np.stack([bs, bh], axis=-1), np.float32)
        in_maps.append(dict(common, x=np.ascontiguousarray(xs),
                            w1=np.ascontiguousarray(w1s),
                            w2=np.ascontiguousarray(w2s), bn2=bn2_t))
    return in_maps


def _get_compiled(debug_stage):
    key = debug_stage
    if key not in _COMPILED:
        _COMPILED[key] = build_kernel(debug_stage)
    return _COMPILED[key]


def run(inputs, debug_stage=None, trace=False, **kwargs):
    nc = _get_compiled(debug_stage)
    in_maps = _prep_inputs(**inputs)
    return run_bass_kernel_spmd(nc, in_maps, list(range(N_CORES)), trace=trace,
                                **kwargs)


def kernel(**inputs):
    res = run(inputs, debug_stage=DEBUG_STAGE)
    out = np.empty((B, C, H, W), np.float32)
    if DEBUG_STAGE is None:
        for core in range(N_CORES):
            b, half = core // 2, core % 2
            out[b, half * 256:(half + 1) * 256] = \
                res.results[core]["out"].reshape(256, H, W)
    else:
        for b in range(B):
            out[b] = res.results[2 * b]["out"].reshape(C, H, W)
    return out
